# revision 33
# baseline (speedup 1.0000x reference)
"""MoE top-2 routing kernel for Trainium2 (8 NeuronCores, expert-parallel).

Strategy (single-pass bf16, SBUF-resident weights):
  - Host: gating (logits/top-2/softmax, fp64 -- 0.05% of total FLOPs), then
    gather each expert's routed tokens into a transposed bf16 batch of
    exactly C=2048 columns (the mean load). Tokens beyond 2048 for
    above-average experts (routing variance, <1% of work) are computed on
    host in fp64, so every core runs an identical, perfectly-balanced
    program with zero padding.
  - Device (SPMD, one expert per core): W1 and W2 fully SBUF-resident in
    bf16 (16.8 MB of 28 MB SBUF). For each 512-column block:
      h = gelu(W1 @ x + b1)   (PSUM accumulation over D, gelu on ScalarE)
      y = W2 @ h              (accumulated over ALL of H in one PSUM bank)
    so y is written to DRAM exactly once, in bf16 (no partials, no re-reads).
    A PE warm-up (dummy matmuls) runs during the ~12us runtime/DMA ramp so
    the HAM clock gate is at 2.4 GHz when real matmuls start.
  - Host: gate-weight scatter-add + b2 combine.

All matmuls run with "feature on partitions, tokens on free dim" layout so no
on-device transposes are needed; host pre-transposes x/W1/W2 (cheap numpy).
"""

import os
import sys
import types

import ml_dtypes
import numpy as np

if "/opt/trn_rl_repo" not in sys.path:
    sys.path.insert(0, "/opt/trn_rl_repo")

import concourse.bacc as bacc
import concourse.mybir as mybir
from concourse.bass_utils import run_bass_kernel_spmd
from concourse.tile import TileContext

P = 128
E = 8
TOP_K = 2
BF16 = ml_dtypes.bfloat16

# Full-problem dims (hardcoded per spec: x (4,2048,1024), W1 (8,4096,1024), ...)
D_FULL = 1024
H_FULL = 4096
DEV_C = 2048  # device columns per expert == mean load (N * TOP_K / E)

LAST_EXEC_TIME_NS = None  # set when MOE_BASS_TRACE=1


def _install_ntff_hook():
    """The image's antenv lacks axon_hooks; inject a shim so trace=True works."""
    if "antenv.axon_hooks" in sys.modules:
        return
    try:
        import antenv.axon_hooks  # noqa: F401  (real module exists, keep it)

        return
    except ImportError:
        pass
    mod = types.ModuleType("antenv.axon_hooks")
    box = [None]
    mod.set_axon_ntff_profile_hook = lambda h: box.__setitem__(0, h)
    mod.get_axon_ntff_profile_hook = lambda: box[0]
    sys.modules["antenv.axon_hooks"] = mod
    try:
        from trn_agent_boot.trn_boot import _ntff_profile_via_ctypes

        mod.set_axon_ntff_profile_hook(
            _ntff_profile_via_ctypes("/opt/axon/libaxon_pjrt.so")
        )
    except Exception:
        pass


def _block_sizes(C, nb):
    """Split C columns into blocks of nb plus >=256 tail blocks (per-MM
    overhead is ~4ns, so keep the moving dim large). C: multiple of 128."""
    assert C % 128 == 0
    n_full, rem = divmod(C, nb)
    out = [nb] * n_full
    if rem == 128:
        assert n_full >= 1
        out = out[:-1] + [384, 256]
    elif rem:
        out.append(rem)
    assert sum(out) == C
    return out


def build_moe_kernel(D, H, C, NB=512):
    """One-expert MoE MLP, single pass: y = W2 @ gelu(W1 @ x + b1), in
    transposed (feature-major) layout, bf16 operands / fp32 accumulation.

    DRAM params per core:
      xt  (D, C)  bf16 : x gathered for this expert, transposed
      w1t (D, H)  bf16 : W1.T
      w2t (H, D)  bf16 : W2.T
      b1t (P, H//P) f32: b1 reshaped so [p, m] = b1[m*P + p]
      y   (D, C)  bf16 out
    """
    f32 = mybir.dt.float32
    bf16 = mybir.dt.bfloat16
    KO = D // P  # contraction subtiles for MM1
    JT = H // P  # j-tiles (h rows / MM2 contraction)
    IT = D // P  # output i-tiles for MM2
    N_WARM = 48  # dummy matmuls covering the DMA ramp (HAM warm-up; >=4.5us
    # of worst-case-cold coverage so real MMs always start at K=8/8)
    blocks = _block_sizes(C, NB)

    nc = bacc.Bacc(None, target_bir_lowering=False)
    xt = nc.declare_dram_parameter("xt", [D, C], bf16, isOutput=False)
    w1t = nc.declare_dram_parameter("w1t", [D, H], bf16, isOutput=False)
    w2t = nc.declare_dram_parameter("w2t", [H, D], bf16, isOutput=False)
    b1t = nc.declare_dram_parameter("b1t", [P, JT], f32, isOutput=False)
    y = nc.declare_dram_parameter("y", [D, C], bf16, isOutput=True)

    xtr = xt.rearrange("(ko p) c -> p ko c", p=P)  # (P, KO, C)
    w1r = w1t.rearrange("(ko p) h -> p ko h", p=P)  # (P, KO, H)
    w2r = w2t.rearrange("(jt p) d -> p jt d", p=P)  # (P, JT, D)
    yr = y.rearrange("(io p) c -> io p c", p=P)  # (IT, P, C)

    gelu = mybir.ActivationFunctionType.Gelu

    with TileContext(nc) as tc:
        with (
            tc.tile_pool(name="w1p", bufs=1) as w1p,
            tc.tile_pool(name="w2p", bufs=1) as w2p,
            tc.tile_pool(name="xp", bufs=3) as xp,
            tc.tile_pool(name="htp", bufs=1) as htp,
            tc.tile_pool(name="ystp", bufs=3) as ystp,
            tc.tile_pool(name="cst", bufs=1) as cst,
            tc.tile_pool(name="psh", bufs=4, space="PSUM") as psh,
            tc.tile_pool(name="psy", bufs=4, space="PSUM") as psy,
        ):
            b1_sb = cst.tile([P, JT], f32)
            w1q = w1p.tile([P, KO, H], bf16, tag="w1q")
            w2q = w2p.tile([P, JT, D], bf16, tag="w2q")

            # PE warm-up: dummy matmuls with no DMA dependency keep the PE
            # busy through the ~12us runtime/DMA ramp so HAM is at K=8/8
            # (2.4 GHz) when the real matmuls start.
            dumw = cst.tile([P, 256], bf16)
            nc.vector.memset(dumw[:], 0.0)
            pwarm = psh.tile([P, NB], f32, tag="ph")
            for _ in range(N_WARM):
                nc.tensor.matmul(
                    pwarm[:, :128], dumw[:, 0:128], dumw[:, 128:256],
                    start=True, stop=True,
                )

            # HWDGE drains one queue FIFO per SDMA engine, so emission order
            # is delivery order: x block 0, W1 in j order (two small chunks
            # first so j-tiles 0..3 unlock after 0.5 MB), b1 off the head
            # path, W2 in i order, then the remaining x blocks.
            xb0 = xp.tile([P, KO, NB], bf16, tag="xb")
            nb0 = blocks[0]
            nc.sync.dma_start(out=xb0[:, 0:2, :nb0], in_=xtr[:, 0:2, 0:nb0])
            nc.sync.dma_start(out=w1q[:, 0:4, 0:256], in_=w1r[:, 0:4, 0:256])
            nc.sync.dma_start(out=xb0[:, 2:4, :nb0], in_=xtr[:, 2:4, 0:nb0])
            nc.sync.dma_start(
                out=w1q[:, 4:KO, 0:256], in_=w1r[:, 4:KO, 0:256]
            )
            nc.sync.dma_start(out=xb0[:, 4:6, :nb0], in_=xtr[:, 4:6, 0:nb0])
            nc.sync.dma_start(
                out=xb0[:, 6:KO, :nb0], in_=xtr[:, 6:KO, 0:nb0]
            )
            nc.sync.dma_start(out=w1q[:, :, 256:512], in_=w1r[:, :, 256:512])
            nc.sync.dma_start(out=b1_sb[:], in_=b1t[:, :])
            for jc in range(512, H, 512):
                nc.sync.dma_start(
                    out=w1q[:, :, jc : jc + 512], in_=w1r[:, :, jc : jc + 512]
                )
            # 512-col W2 chunks: 1 KB contiguous runs (>=512 B keeps SDMA at
            # line rate; 128-col chunks' 256 B runs measured ~2x slower).
            for ic in range(0, D, 512):
                nc.sync.dma_start(
                    out=w2q[:, :, ic : ic + 512], in_=w2r[:, :, ic : ic + 512]
                )

            col = 0
            for b, nb in enumerate(blocks):
                if b == 0:
                    xb = xb0
                else:
                    xb = xp.tile([P, KO, NB], bf16, tag="xb")
                    nc.sync.dma_start(
                        out=xb[:, :, :nb], in_=xtr[:, :, col : col + nb]
                    )
                ht = htp.tile([P, JT, NB], bf16, tag="ht")
                # MM1: h^T[j, t] = sum_d W1[j, d] x[t, d], then gelu+bias
                for j in range(JT):
                    ph = psh.tile([P, NB], f32, tag="ph")
                    for k in range(KO):
                        nc.tensor.matmul(
                            ph[:, :nb],
                            w1q[:, k, j * P : (j + 1) * P],
                            xb[:, k, :nb],
                            start=(k == 0),
                            stop=(k == KO - 1),
                        )
                    nc.scalar.activation(
                        ht[:, j, :nb],
                        ph[:, :nb],
                        gelu,
                        bias=b1_sb[:, j : j + 1],
                        scale=1.0,
                    )
                # MM2: y^T[i, t] = sum_j W2[i, j] h^T[j, t] over ALL of H
                for i in range(IT):
                    py = psy.tile([P, NB], f32, tag="py")
                    for j in range(JT):
                        nc.tensor.matmul(
                            py[:, :nb],
                            w2q[:, j, i * P : (i + 1) * P],
                            ht[:, j, :nb],
                            start=(j == 0),
                            stop=(j == JT - 1),
                        )
                    st = ystp.tile([P, NB], bf16, tag="st")
                    nc.vector.tensor_copy(out=st[:, :nb], in_=py[:, :nb])
                    nc.sync.dma_start(
                        out=yr[i, :, col : col + nb], in_=st[:, :nb]
                    )
                col += nb
    nc.finalize()
    return nc


_kernel_cache = {}


def _get_kernel(D, H, C, NB=512):
    key = (D, H, C, NB)
    if key not in _kernel_cache:
        _kernel_cache[key] = build_moe_kernel(D, H, C, NB)
    return _kernel_cache[key]


def _topk_gating(xf, Wg):
    """Replicate jax.lax.top_k(logits, 2) + softmax in fp64 on host."""
    logits = xf.astype(np.float64) @ Wg.T.astype(np.float64)  # (N, E)
    order = np.argsort(-logits, axis=1, kind="stable")[:, :TOP_K]  # (N, 2)
    top = np.take_along_axis(logits, order, axis=1)  # (N, 2) descending
    m = top.max(axis=1, keepdims=True)
    e = np.exp(top - m)
    w = e / e.sum(axis=1, keepdims=True)  # (N, 2)
    return order, w


def _gelu_exact(x):
    """Exact gelu x * Phi(x) for fp64 input via A&S 7.1.26 erf (|err|<1.5e-7)."""
    z = x / np.sqrt(2.0)
    s = np.sign(z)
    a = np.abs(z)
    t = 1.0 / (1.0 + 0.3275911 * a)
    poly = t * (
        0.254829592
        + t * (-0.284496736 + t * (1.421413741 + t * (-1.453152027 + t * 1.061405429)))
    )
    erf = s * (1.0 - poly * np.exp(-a * a))
    return x * 0.5 * (1.0 + erf)


def kernel(x, Wg, W1, b1, W2, b2):
    global LAST_EXEC_TIME_NS
    x = np.asarray(x, dtype=np.float32)
    Wg = np.asarray(Wg, dtype=np.float32)
    W1 = np.asarray(W1, dtype=np.float32)
    b1 = np.asarray(b1, dtype=np.float32)
    W2 = np.asarray(W2, dtype=np.float32)
    b2 = np.asarray(b2, dtype=np.float32)
    B, T, D = x.shape
    H = W1.shape[1]
    N = B * T
    xf = np.ascontiguousarray(x.reshape(N, D), dtype=np.float32)

    top_idx, top_w = _topk_gating(xf, Wg)

    # Per-expert routed token lists + gate weights
    ids = []
    gws = []
    for e in range(E):
        hit = top_idx == e  # (N, 2)
        sel = hit.any(axis=1)
        ids_e = np.nonzero(sel)[0]
        w_e = np.where(hit[ids_e, 0], top_w[ids_e, 0], top_w[ids_e, 1])
        ids.append(ids_e)
        gws.append(w_e.astype(np.float32))

    C = DEV_C
    nc = _get_kernel(D, H, C)

    in_maps = []
    for e in range(E):
        xt = np.zeros((D, C), dtype=BF16)
        cnt = min(len(ids[e]), C)
        xt[:, :cnt] = xf[ids[e][:cnt]].T.astype(BF16)
        in_maps.append(
            {
                "xt": xt,
                "w1t": np.ascontiguousarray(W1[e].T).astype(BF16),
                "w2t": np.ascontiguousarray(W2[e].T).astype(BF16),
                "b1t": np.ascontiguousarray(
                    np.asarray(b1[e], dtype=np.float32).reshape(H // P, P).T
                ),
            }
        )

    trace = os.environ.get("MOE_BASS_TRACE", "0") == "1"
    # Install the profile-hook shim unconditionally: run_bass_kernel_spmd also
    # enables tracing when BASS_TRACE is set in the environment.
    _install_ntff_hook()
    res = run_bass_kernel_spmd(nc, in_maps, core_ids=list(range(E)), trace=trace)
    if trace:
        LAST_EXEC_TIME_NS = res.exec_time_ns

    out = np.zeros((N, D), dtype=np.float32)
    for e in range(E):
        cnt = min(len(ids[e]), C)
        if cnt:
            y_e = np.asarray(res.results[e]["y"], dtype=np.float32)  # (D, C)
            out[ids[e][:cnt]] += gws[e][:cnt, None] * y_e[:, :cnt].T
        if len(ids[e]) > C:
            # Routing-variance overflow: tokens beyond the mean load, exact
            # on host in fp64 (<1% of total FLOPs).
            ov = ids[e][C:]
            xo = xf[ov].astype(np.float64)
            h = _gelu_exact(xo @ W1[e].T.astype(np.float64) + b1[e].astype(np.float64))
            yo = h @ W2[e].T.astype(np.float64)
            out[ov] += gws[e][C:, None] * yo.astype(np.float32)

    # b2 combine: sum_k w_k * b2[e_k] per token
    w_dense = np.zeros((N, E), dtype=np.float32)
    np.put_along_axis(w_dense, top_idx, top_w.astype(np.float32), axis=1)
    out += w_dense @ np.asarray(b2, dtype=np.float32)

    return out.reshape(B, T, D).astype(np.float32)


# revision 34
# speedup vs baseline: 1.0053x; 1.0053x over previous
"""MoE top-2 routing kernel for Trainium2 (8 NeuronCores, expert-parallel).

Strategy (single-pass bf16, SBUF-resident weights):
  - Host: gating (logits/top-2/softmax, fp64 -- 0.05% of total FLOPs), then
    gather each expert's routed tokens into a transposed bf16 batch of
    exactly C=2048 columns (the mean load). Tokens beyond 2048 for
    above-average experts (routing variance, <1% of work) are computed on
    host in fp64, so every core runs an identical, perfectly-balanced
    program with zero padding.
  - Device (SPMD, one expert per core): W1 and W2 fully SBUF-resident in
    bf16 (16.8 MB of 28 MB SBUF). For each 512-column block:
      h = gelu(W1 @ x + b1)   (PSUM accumulation over D, gelu on ScalarE)
      y = W2 @ h              (accumulated over ALL of H in one PSUM bank)
    so y is written to DRAM exactly once, in bf16 (no partials, no re-reads).
    A PE warm-up (dummy matmuls) runs during the ~12us runtime/DMA ramp so
    the HAM clock gate is at 2.4 GHz when real matmuls start.
  - Host: gate-weight scatter-add + b2 combine.

All matmuls run with "feature on partitions, tokens on free dim" layout so no
on-device transposes are needed; host pre-transposes x/W1/W2 (cheap numpy).
"""

import os
import sys
import types

import ml_dtypes
import numpy as np

if "/opt/trn_rl_repo" not in sys.path:
    sys.path.insert(0, "/opt/trn_rl_repo")

import concourse.bacc as bacc
import concourse.mybir as mybir
from concourse.bass_utils import run_bass_kernel_spmd
from concourse.tile import TileContext

P = 128
E = 8
TOP_K = 2
BF16 = ml_dtypes.bfloat16

# Full-problem dims (hardcoded per spec: x (4,2048,1024), W1 (8,4096,1024), ...)
D_FULL = 1024
H_FULL = 4096
DEV_C = 2048  # device columns per expert == mean load (N * TOP_K / E)

LAST_EXEC_TIME_NS = None  # set when MOE_BASS_TRACE=1


def _install_ntff_hook():
    """The image's antenv lacks axon_hooks; inject a shim so trace=True works."""
    if "antenv.axon_hooks" in sys.modules:
        return
    try:
        import antenv.axon_hooks  # noqa: F401  (real module exists, keep it)

        return
    except ImportError:
        pass
    mod = types.ModuleType("antenv.axon_hooks")
    box = [None]
    mod.set_axon_ntff_profile_hook = lambda h: box.__setitem__(0, h)
    mod.get_axon_ntff_profile_hook = lambda: box[0]
    sys.modules["antenv.axon_hooks"] = mod
    try:
        from trn_agent_boot.trn_boot import _ntff_profile_via_ctypes

        mod.set_axon_ntff_profile_hook(
            _ntff_profile_via_ctypes("/opt/axon/libaxon_pjrt.so")
        )
    except Exception:
        pass


def _block_sizes(C, nb):
    """Split C columns into blocks of nb plus >=256 tail blocks (per-MM
    overhead is ~4ns, so keep the moving dim large). C: multiple of 128."""
    assert C % 128 == 0
    n_full, rem = divmod(C, nb)
    out = [nb] * n_full
    if rem == 128:
        assert n_full >= 1
        out = out[:-1] + [384, 256]
    elif rem:
        out.append(rem)
    assert sum(out) == C
    return out


def build_moe_kernel(D, H, C, NB=512):
    """One-expert MoE MLP, single pass: y = W2 @ gelu(W1 @ x + b1), in
    transposed (feature-major) layout, bf16 operands / fp32 accumulation.

    DRAM params per core:
      xt  (D, C)  bf16 : x gathered for this expert, transposed
      w1t (D, H)  bf16 : W1.T
      w2t (H, D)  bf16 : W2.T
      b1t (P, H//P) f32: b1 reshaped so [p, m] = b1[m*P + p]
      y   (D, C)  bf16 out
    """
    f32 = mybir.dt.float32
    bf16 = mybir.dt.bfloat16
    KO = D // P  # contraction subtiles for MM1
    JT = H // P  # j-tiles (h rows / MM2 contraction)
    IT = D // P  # output i-tiles for MM2
    N_WARM = 80  # dummy matmuls covering the ~13.6us DMA ramp (HAM warm-up)
    blocks = _block_sizes(C, NB)

    nc = bacc.Bacc(None, target_bir_lowering=False)
    xt = nc.declare_dram_parameter("xt", [D, C], bf16, isOutput=False)
    w1t = nc.declare_dram_parameter("w1t", [D, H], bf16, isOutput=False)
    w2t = nc.declare_dram_parameter("w2t", [H, D], bf16, isOutput=False)
    b1t = nc.declare_dram_parameter("b1t", [P, JT], f32, isOutput=False)
    y = nc.declare_dram_parameter("y", [D, C], bf16, isOutput=True)

    xtr = xt.rearrange("(ko p) c -> p ko c", p=P)  # (P, KO, C)
    w1r = w1t.rearrange("(ko p) h -> p ko h", p=P)  # (P, KO, H)
    w2r = w2t.rearrange("(jt p) d -> p jt d", p=P)  # (P, JT, D)
    yr = y.rearrange("(io p) c -> io p c", p=P)  # (IT, P, C)

    gelu = mybir.ActivationFunctionType.Gelu

    with TileContext(nc) as tc:
        with (
            tc.tile_pool(name="w1p", bufs=1) as w1p,
            tc.tile_pool(name="w2p", bufs=1) as w2p,
            tc.tile_pool(name="xp", bufs=3) as xp,
            tc.tile_pool(name="htp", bufs=1) as htp,
            tc.tile_pool(name="ystp", bufs=3) as ystp,
            tc.tile_pool(name="cst", bufs=1) as cst,
            tc.tile_pool(name="psh", bufs=4, space="PSUM") as psh,
            tc.tile_pool(name="psy", bufs=4, space="PSUM") as psy,
        ):
            b1_sb = cst.tile([P, JT], f32)
            w1q = w1p.tile([P, KO, H], bf16, tag="w1q")
            w2q = w2p.tile([P, JT, D], bf16, tag="w2q")

            # PE warm-up: dummy matmuls with no DMA dependency keep the PE
            # busy through the ~12us runtime/DMA ramp so HAM is at K=8/8
            # (2.4 GHz) when the real matmuls start.
            dumw = cst.tile([P, 256], bf16)
            nc.vector.memset(dumw[:], 0.0)
            pwarm = psh.tile([P, NB], f32, tag="ph")
            for _ in range(N_WARM):
                nc.tensor.matmul(
                    pwarm[:, :128], dumw[:, 0:128], dumw[:, 128:256],
                    start=True, stop=True,
                )

            # HWDGE drains one queue FIFO per SDMA engine, so emission order
            # is delivery order: x block 0, W1 in j order (two small chunks
            # first so j-tiles 0..3 unlock after 0.5 MB), b1 off the head
            # path, W2 in i order, then the remaining x blocks.
            xb0 = xp.tile([P, KO, NB], bf16, tag="xb")
            nb0 = blocks[0]
            nc.sync.dma_start(out=xb0[:, :, :nb0], in_=xtr[:, :, 0:nb0])
            w1_chunks = [(0, 256), (256, 256)] + [
                (jc, 512) for jc in range(512, H, 512)
            ]
            for ci, (jc, w) in enumerate(w1_chunks):
                nc.sync.dma_start(
                    out=w1q[:, :, jc : jc + w], in_=w1r[:, :, jc : jc + w]
                )
                if ci == 0:
                    nc.sync.dma_start(out=b1_sb[:], in_=b1t[:, :])
            # 512-col W2 chunks: 1 KB contiguous runs (>=512 B keeps SDMA at
            # line rate; 128-col chunks' 256 B runs measured ~2x slower).
            for ic in range(0, D, 512):
                nc.sync.dma_start(
                    out=w2q[:, :, ic : ic + 512], in_=w2r[:, :, ic : ic + 512]
                )

            col = 0
            for b, nb in enumerate(blocks):
                if b == 0:
                    xb = xb0
                else:
                    xb = xp.tile([P, KO, NB], bf16, tag="xb")
                    nc.sync.dma_start(
                        out=xb[:, :, :nb], in_=xtr[:, :, col : col + nb]
                    )
                ht = htp.tile([P, JT, NB], bf16, tag="ht")
                # MM1: h^T[j, t] = sum_d W1[j, d] x[t, d], then gelu+bias
                for j in range(JT):
                    ph = psh.tile([P, NB], f32, tag="ph")
                    for k in range(KO):
                        nc.tensor.matmul(
                            ph[:, :nb],
                            w1q[:, k, j * P : (j + 1) * P],
                            xb[:, k, :nb],
                            start=(k == 0),
                            stop=(k == KO - 1),
                        )
                    nc.scalar.activation(
                        ht[:, j, :nb],
                        ph[:, :nb],
                        gelu,
                        bias=b1_sb[:, j : j + 1],
                        scale=1.0,
                    )
                # MM2: y^T[i, t] = sum_j W2[i, j] h^T[j, t] over ALL of H
                for i in range(IT):
                    py = psy.tile([P, NB], f32, tag="py")
                    for j in range(JT):
                        nc.tensor.matmul(
                            py[:, :nb],
                            w2q[:, j, i * P : (i + 1) * P],
                            ht[:, j, :nb],
                            start=(j == 0),
                            stop=(j == JT - 1),
                        )
                    st = ystp.tile([P, NB], bf16, tag="st")
                    nc.vector.tensor_copy(out=st[:, :nb], in_=py[:, :nb])
                    nc.sync.dma_start(
                        out=yr[i, :, col : col + nb], in_=st[:, :nb]
                    )
                col += nb
    nc.finalize()
    return nc


_kernel_cache = {}


def _get_kernel(D, H, C, NB=512):
    key = (D, H, C, NB)
    if key not in _kernel_cache:
        _kernel_cache[key] = build_moe_kernel(D, H, C, NB)
    return _kernel_cache[key]


def _topk_gating(xf, Wg):
    """Replicate jax.lax.top_k(logits, 2) + softmax in fp64 on host."""
    logits = xf.astype(np.float64) @ Wg.T.astype(np.float64)  # (N, E)
    order = np.argsort(-logits, axis=1, kind="stable")[:, :TOP_K]  # (N, 2)
    top = np.take_along_axis(logits, order, axis=1)  # (N, 2) descending
    m = top.max(axis=1, keepdims=True)
    e = np.exp(top - m)
    w = e / e.sum(axis=1, keepdims=True)  # (N, 2)
    return order, w


def _gelu_exact(x):
    """Exact gelu x * Phi(x) for fp64 input via A&S 7.1.26 erf (|err|<1.5e-7)."""
    z = x / np.sqrt(2.0)
    s = np.sign(z)
    a = np.abs(z)
    t = 1.0 / (1.0 + 0.3275911 * a)
    poly = t * (
        0.254829592
        + t * (-0.284496736 + t * (1.421413741 + t * (-1.453152027 + t * 1.061405429)))
    )
    erf = s * (1.0 - poly * np.exp(-a * a))
    return x * 0.5 * (1.0 + erf)


def kernel(x, Wg, W1, b1, W2, b2):
    global LAST_EXEC_TIME_NS
    x = np.asarray(x, dtype=np.float32)
    Wg = np.asarray(Wg, dtype=np.float32)
    W1 = np.asarray(W1, dtype=np.float32)
    b1 = np.asarray(b1, dtype=np.float32)
    W2 = np.asarray(W2, dtype=np.float32)
    b2 = np.asarray(b2, dtype=np.float32)
    B, T, D = x.shape
    H = W1.shape[1]
    N = B * T
    xf = np.ascontiguousarray(x.reshape(N, D), dtype=np.float32)

    top_idx, top_w = _topk_gating(xf, Wg)

    # Per-expert routed token lists + gate weights
    ids = []
    gws = []
    for e in range(E):
        hit = top_idx == e  # (N, 2)
        sel = hit.any(axis=1)
        ids_e = np.nonzero(sel)[0]
        w_e = np.where(hit[ids_e, 0], top_w[ids_e, 0], top_w[ids_e, 1])
        ids.append(ids_e)
        gws.append(w_e.astype(np.float32))

    C = DEV_C
    nc = _get_kernel(D, H, C)

    in_maps = []
    for e in range(E):
        xt = np.zeros((D, C), dtype=BF16)
        cnt = min(len(ids[e]), C)
        xt[:, :cnt] = xf[ids[e][:cnt]].T.astype(BF16)
        in_maps.append(
            {
                "xt": xt,
                "w1t": np.ascontiguousarray(W1[e].T).astype(BF16),
                "w2t": np.ascontiguousarray(W2[e].T).astype(BF16),
                "b1t": np.ascontiguousarray(
                    np.asarray(b1[e], dtype=np.float32).reshape(H // P, P).T
                ),
            }
        )

    trace = os.environ.get("MOE_BASS_TRACE", "0") == "1"
    # Install the profile-hook shim unconditionally: run_bass_kernel_spmd also
    # enables tracing when BASS_TRACE is set in the environment.
    _install_ntff_hook()
    res = run_bass_kernel_spmd(nc, in_maps, core_ids=list(range(E)), trace=trace)
    if trace:
        LAST_EXEC_TIME_NS = res.exec_time_ns

    out = np.zeros((N, D), dtype=np.float32)
    for e in range(E):
        cnt = min(len(ids[e]), C)
        if cnt:
            y_e = np.asarray(res.results[e]["y"], dtype=np.float32)  # (D, C)
            out[ids[e][:cnt]] += gws[e][:cnt, None] * y_e[:, :cnt].T
        if len(ids[e]) > C:
            # Routing-variance overflow: tokens beyond the mean load, exact
            # on host in fp64 (<1% of total FLOPs).
            ov = ids[e][C:]
            xo = xf[ov].astype(np.float64)
            h = _gelu_exact(xo @ W1[e].T.astype(np.float64) + b1[e].astype(np.float64))
            yo = h @ W2[e].T.astype(np.float64)
            out[ov] += gws[e][C:, None] * yo.astype(np.float32)

    # b2 combine: sum_k w_k * b2[e_k] per token
    w_dense = np.zeros((N, E), dtype=np.float32)
    np.put_along_axis(w_dense, top_idx, top_w.astype(np.float32), axis=1)
    out += w_dense @ np.asarray(b2, dtype=np.float32)

    return out.reshape(B, T, D).astype(np.float32)
